# revision 24
# baseline (speedup 1.0000x reference)
"""Trainium2 Bass kernel for retrieval-knn attention classifier (nn_MA_51866025067137).

Strategy (8 NeuronCores):
  Phase 1 — memory_keys sharded along N (12800 keys/core, padded 100000->102400).
  Keys/queries are quantized to small integers and fed to fp8e4 DoubleRow
  matmuls (0.5 cycles/row on the PE).  Six contraction slots (dims 381-383,
  509-511) are repurposed to add BIG + iota*2^-9 inside the same matmul, so
  every similarity lands in one fp32 binade [2^14, 2^15) with its low 10
  mantissa bits equal to the in-window column index (sims are exact even
  integers, so the index pack costs nothing; each DoubleRow (i0,i1) pair-sum
  is kept fp16-exact because the PE reduces pairs in half precision).
  Per-1024-window top-8 is then a single DVE max8 straight out of PSUM;
  6400 of the 12800 columns instead go through ACT eviction + two GPSIMD
  topk calls (top-256 per 16-partition token, with indices), interleaved
  window-by-window so ACT/DVE/Pool/PE/DMA all run concurrently.  Candidate
  merging + exact re-scoring on the host yields the exact global top-32.
  Phase 2 — batch sharded (32 queries/core): memory-attention module
  (tanh(qWq + knnWm + b) -> scores -> softmax -> weighted sum) + classifier,
  in bf16 with fp32 PSUM; qWq is folded into the knnWm PSUM group via a
  selector matmul, softmax renormalization is folded into the final output
  combine, and the [1,1024]->[128,8] softmax redistribution uses PE
  transposes instead of a DRAM bounce.
"""

import numpy as np
import ml_dtypes

import concourse.bacc as bacc
import concourse.mybir as mybir
from concourse.tile import TileContext, add_dep_helper
from concourse.bass_utils import run_bass_kernel_spmd
from concourse.masks import make_identity

# problem dims (hardcoded per harness contract)
B, N, D = 256, 100000, 512
A, C, K = 256, 100, 32
NC_CORES = 8
NPAD = 102400             # 8 * 12800
SHARD = NPAD // NC_CORES  # 12800
WIN = 1024                # DVE max8 window
NWIN = 13                 # windows 0..11 full, 12 is half (512)
TOPW = 3200               # gpsimd topk region width (vocab = 16*3200 = 51200)
NREG = 2                  # topk regions per qt: cols [0, 6400)
TOPC = NREG * TOPW        # 6400 cols to ACT-evict per qt
BROWS = B // NC_CORES     # 32 rows per core in phase 2
BIG = 24576.0             # binade [2^14, 2^15); ulp 2^-9
ULP = 2.0 ** -9

f32 = mybir.dt.float32
f32r = mybir.dt.float32r
f8 = mybir.dt.float8e4
u32 = mybir.dt.uint32
bf16 = mybir.dt.bfloat16

_PH1 = None
_PH2 = None

# ---- phase-1 window plan: interleave ACT-evict (topk) and DVE-max8 windows
# so the two consumer engines run concurrently.  Evict pieces fill the two
# topk region buffers sequentially; max8 windows append 8 cols each to L1.
_EV_SRC = [(0, 0, 1024), (1, 0, 256), (2, 0, 1024), (4, 0, 1024),
           (6, 0, 1024), (8, 0, 1024), (10, 0, 1024)]   # (window, lo, len)
_MAX8_LIST = [(1, 256, 1024), (3, 0, 1024), (5, 0, 1024), (7, 0, 1024),
              (9, 0, 1024), (11, 0, 1024), (12, 0, 512)]  # (window, lo, hi)


def _ev_pieces():
    """(window, src_lo, src_hi, region, dst_off) with region splits applied."""
    out = []
    dst = 0
    for (w, lo, ln) in _EV_SRC:
        left = ln
        src = lo
        while left:
            r = dst // TOPW
            take = min(left, (r + 1) * TOPW - dst)
            out.append((w, src, src + take, r, dst - r * TOPW))
            dst += take
            src += take
            left -= take
    assert dst == TOPC
    return out


_EV_PIECES = _ev_pieces()



def _u(i):
    return i.ins if hasattr(i, "ins") else i


def _build_phase1():
    nc = bacc.Bacc("TRN2", target_bir_lowering=False)
    kT_d = nc.dram_tensor("kT", [13, 128, 2, 2, 2, 512], f8, kind="ExternalInput")
    qT_d = nc.dram_tensor("qT", [2, 128, 2, 2, 128], f8, kind="ExternalInput")
    l1_d = nc.dram_tensor("l1", [2, 128, 56], f32, kind="ExternalOutput")
    tk_d = nc.dram_tensor("tk", [2, NREG, 128, 32], u32, kind="ExternalOutput")

    with TileContext(nc) as tc:
        with (
            tc.tile_pool(name="const", bufs=1) as constp,
            tc.tile_pool(name="keys", bufs=6) as keyp,
            tc.tile_pool(name="l1", bufs=1) as l1p,
            tc.tile_pool(name="psum", bufs=2, space="PSUM") as psump,
        ):
            qT = [constp.tile([128, 2, 2, 128], f8, tag=f"qT{qt}", name=f"qT_t{qt}")
                  for qt in range(2)]
            for qt in range(2):
                nc.sync.dma_start(out=qT[qt][:], in_=qT_d[qt, :, :, :, :])

            # raw SBUF for gpsimd topk (per qt) + its output
            sims_sb = [[nc.alloc_sbuf_tensor(f"sims_sb{qt}_{r}", [128, TOPW], f32)
                        for r in range(NREG)] for qt in range(2)]
            tk_sb = [[nc.alloc_sbuf_tensor(f"tk_sb{qt}_{r}", [128, 32], u32)
                      for r in range(NREG)] for qt in range(2)]

            L1 = [l1p.tile([128, 56], f32, tag=f"l1_{qt}", name=f"l1_{qt}")
                  for qt in range(2)]
            evicts = [[[], []], [[], []]]  # [qt][region]: ACT evicts feeding topk

            ev_by_w = {}
            for (w, lo, hi, r, doff) in _EV_PIECES:
                ev_by_w.setdefault(w, []).append((lo, hi, r, doff))
            max8_by_w = {w: (lo, hi) for (w, lo, hi) in _MAX8_LIST}
            l1_off = {w: 8 * i for i, (w, lo, hi) in enumerate(_MAX8_LIST)}

            for w in range(NWIN):
                wcols = 512 if w == 12 else WIN
                nchunk = wcols // 512
                kt = keyp.tile([128, 2, 2, 2, 512], f8, tag="kt", name="kt_t")
                nc.sync.dma_start(out=kt[:], in_=kT_d[w, :, :, :, :, :])
                for qt in range(2):
                    ps = psump.tile([128, WIN], f32, tag=f"win{qt}", name=f"ps{qt}")
                    for h in range(nchunk):
                        sl = slice(h * 512, (h + 1) * 512)
                        for dc in range(2):
                            nc.tensor.matmul(
                                ps[:, sl],
                                lhsT=qT[qt][:, dc, :, :],
                                rhs=kt[:, h, dc, :, :],
                                start=(dc == 0), stop=(dc == 1),
                                perf_mode=mybir.MatmulPerfMode.DoubleRow)
                    for (lo, hi, r, doff) in ev_by_w.get(w, []):
                        ev = nc.scalar.copy(
                            out=sims_sb[qt][r][:, doff:doff + hi - lo],
                            in_=ps[:, lo:hi])
                        evicts[qt][r].append(ev)
                    if w in max8_by_w:
                        lo, hi = max8_by_w[w]
                        o = l1_off[w]
                        nc.vector.max(out=L1[qt][:, o:o + 8], in_=ps[:, lo:hi])

            for qt in range(2):
                for r in range(NREG):
                    tki = nc.gpsimd.topk(
                        tk_sb[qt][r][:], sims_sb[qt][r][:],
                        tokens=8, vocab_size=16 * TOPW, k=256)
                    for ev in evicts[qt][r]:
                        add_dep_helper(_u(tki), _u(ev), reason="topk waits evicts")
                    do = nc.sync.dma_start(out=tk_d[qt, r, :, :],
                                           in_=tk_sb[qt][r][:])
                    add_dep_helper(_u(do), _u(tki), reason="tk out waits topk")
                nc.sync.dma_start(out=l1_d[qt, :, :], in_=L1[qt][:])
    nc.finalize()
    return nc


def _build_phase2():
    nc = bacc.Bacc("TRN2", target_bir_lowering=False)
    NCD = BROWS * K  # 1024
    qTr_in = nc.dram_tensor("qTr", [D, BROWS], bf16, kind="ExternalInput")    # relu'd
    knnT_in = nc.dram_tensor("knnT", [D, NCD], bf16, kind="ExternalInput")
    knn_in = nc.dram_tensor("knn", [NCD, D], bf16, kind="ExternalInput")
    Wqm_in = nc.dram_tensor("Wqm", [D, 2 * A], bf16, kind="ExternalInput")
    Ws_in = nc.dram_tensor("Ws", [A, 1], bf16, kind="ExternalInput")
    bqm_in = nc.dram_tensor("bqm", [A, 1], f32, kind="ExternalInput")         # bq+bm
    Wc_in = nc.dram_tensor("Wc", [2 * D, C], bf16, kind="ExternalInput")
    S_in = nc.dram_tensor("S", [BROWS, NCD], bf16, kind="ExternalInput")      # S[b,(b',k)]=d_bb'
    m256_in = nc.dram_tensor("m256", [128, 256], bf16, kind="ExternalInput")
    out_d = nc.dram_tensor("out", [BROWS, C], f32, kind="ExternalOutput")     # +bc host

    with TileContext(nc) as tc:
        with (
            tc.tile_pool(name="big", bufs=1) as bigp,
            tc.tile_pool(name="small", bufs=1) as smallp,
            tc.tile_pool(name="psum", bufs=1, space="PSUM") as psump,
        ):
            # ---- loads, ordered for earliest compute start ----
            qTr = smallp.tile([128, 4, BROWS], bf16, tag="qTr")
            nc.sync.dma_start(out=qTr[:],
                              in_=qTr_in[:].rearrange("(dc p) b -> p dc b", p=128))
            Wqmall = smallp.tile([128, 4, 2 * A], bf16, tag="Wqmall")
            nc.sync.dma_start(out=Wqmall[:],
                              in_=Wqm_in[:].rearrange("(dc p) a -> p dc a", p=128))
            Wqall = Wqmall[:, :, :A]
            Wmall = Wqmall[:, :, A:]
            knnTall = bigp.tile([128, 4, NCD], bf16, tag="knnTall")
            for dc in range(4):
                nc.sync.dma_start(out=knnTall[:, dc, :],
                                  in_=knnT_in[dc * 128:(dc + 1) * 128, :])
            S = smallp.tile([BROWS, NCD], bf16, tag="S")
            nc.sync.dma_start(out=S[:], in_=S_in[:, :])
            Ws = [smallp.tile([128, 1], bf16, tag=f"Ws{at}", name=f"Wst{at}")
                  for at in range(2)]
            bqm = [smallp.tile([128, 1], f32, tag=f"bqm{at}", name=f"bqmt{at}")
                   for at in range(2)]
            for at in range(2):
                nc.sync.dma_start(out=Ws[at][:], in_=Ws_in[at * 128:(at + 1) * 128, :])
                nc.sync.dma_start(out=bqm[at][:], in_=bqm_in[at * 128:(at + 1) * 128, :])
            m256 = smallp.tile([128, 256], bf16, tag="m256")
            nc.sync.dma_start(out=m256[:], in_=m256_in[:, :])
            knnall = bigp.tile([128, 8, D], bf16, tag="knnall")
            nc.sync.dma_start(out=knnall[:],
                              in_=knn_in[:].rearrange("(t p) d -> p t d", p=128))
            Wcall = smallp.tile([128, 8, C], bf16, tag="Wcall")
            nc.sync.dma_start(out=Wcall[:],
                              in_=Wc_in[:].rearrange("(m p) j -> p m j", p=128))
            ones = smallp.tile([128, 1], bf16, tag="ones")
            nc.vector.memset(ones[:].bitcast(mybir.dt.uint16), 0x3F80)
            ident = smallp.tile([128, 128], f32, tag="ident")
            make_identity(nc, ident[:])

            # ---- qproj [32b, 256a] = qTr.T @ Wq ----
            qp_ps = psump.tile([BROWS, A], f32, tag="ps_misc", bufs=2, name="qp_ps")
            for dc in range(4):
                nc.tensor.matmul(qp_ps[:], lhsT=qTr[:, dc, :], rhs=Wqall[:, dc, :],
                                 start=(dc == 0), stop=(dc == 3))
            qproj = smallp.tile([BROWS, A], bf16, tag="qproj")
            nc.vector.tensor_scalar_mul(qproj[:], qp_ps[:], 1.0)

            # ---- h^T = tanh(Wm.T knn.T + qproj via S + bqm); scores ----
            sc_ps = psump.tile([1, NCD], f32, tag="ps_sc")
            hT = [bigp.tile([128, NCD], bf16, tag=f"hT{at}", name=f"hTt{at}")
                  for at in range(2)]
            for at in range(2):
                kp = psump.tile([128, NCD], f32, tag="ps_kp", bufs=2)
                for dc in range(4):
                    for half in range(2):
                        nc.tensor.matmul(
                            kp[:, half * 512:(half + 1) * 512],
                            lhsT=Wmall[:, dc, at * 128:(at + 1) * 128],
                            rhs=knnTall[:, dc, half * 512:(half + 1) * 512],
                            start=(dc == 0), stop=False)
                for half in range(2):
                    nc.tensor.matmul(
                        kp[:, half * 512:(half + 1) * 512],
                        lhsT=qproj[:, at * 128:(at + 1) * 128],
                        rhs=S[:, half * 512:(half + 1) * 512],
                        start=False, stop=(half == 1))
                nc.scalar.activation(hT[at][:], kp[:],
                                     mybir.ActivationFunctionType.Tanh,
                                     bias=bqm[at][:])
                for half in range(2):
                    nc.tensor.matmul(
                        sc_ps[:, half * 512:(half + 1) * 512],
                        lhsT=Ws[at][:],
                        rhs=hT[at][:, half * 512:(half + 1) * 512],
                        start=(at == 0), stop=(at == 1))
            e_row = smallp.tile([1, NCD], f32, tag="e_row")
            nc.scalar.activation(e_row[:], sc_ps[:1, :],
                                 mybir.ActivationFunctionType.Exp)
            # [1, 1024] -> [128, 8] via 8 PE transposes
            ec_ps = psump.tile([128, 8], f32, tag="ps_misc", bufs=2, name="ec_ps")
            for t in range(8):
                nc.tensor.transpose(ec_ps[:, t:t + 1], e_row[:1, t * 128:(t + 1) * 128],
                                    ident[:1, :1])

            # ---- w2[p, (t,j)] = e_col[p, t] * m256 ; den; attT ----
            w2 = bigp.tile([128, 256], bf16, tag="w2")
            eb = ec_ps[:, :, None].to_broadcast([128, 8, 32])
            nc.vector.tensor_tensor(w2[:].rearrange("p (t j) -> p t j", t=8),
                                    m256[:].rearrange("p (t j) -> p t j", t=8),
                                    eb, mybir.AluOpType.mult)
            den_ps = psump.tile([BROWS, 1], f32, tag="ps_misc", bufs=2, name="den_ps")
            for t in range(8):
                nc.tensor.matmul(den_ps[:], lhsT=w2[:, t * 32:(t + 1) * 32],
                                 rhs=ones[:], start=(t == 0), stop=(t == 7))
            rden = smallp.tile([BROWS, 1], f32, tag="rden")
            nc.vector.reciprocal(rden[:], den_ps[:])
            attT = smallp.tile([128, 4, BROWS], bf16, tag="attT")
            for dc in range(4):
                att_ps = psump.tile([128, BROWS], f32, tag="ps_misc", bufs=2,
                                    name=f"att_ps{dc}")
                for t in range(8):
                    nc.tensor.matmul(att_ps[:],
                                     lhsT=knnall[:, t, dc * 128:(dc + 1) * 128],
                                     rhs=w2[:, t * 32:(t + 1) * 32],
                                     start=(t == 0), stop=(t == 7))
                nc.vector.tensor_scalar_mul(attT[:, dc, :], att_ps[:], 1.0)

            # ---- classifier: out = q-part + rden * att-part (att unnormalized) ----
            out1 = psump.tile([BROWS, C], f32, tag="ps_misc", bufs=2, name="out1")
            out2 = psump.tile([BROWS, C], f32, tag="ps_misc", bufs=2, name="out2")
            for dc in range(4):
                nc.tensor.matmul(out1[:], lhsT=qTr[:, dc, :], rhs=Wcall[:, dc, :],
                                 start=(dc == 0), stop=(dc == 3))
                nc.tensor.matmul(out2[:], lhsT=attT[:, dc, :], rhs=Wcall[:, 4 + dc, :],
                                 start=(dc == 0), stop=(dc == 3))
            out1_sb = smallp.tile([BROWS, C], f32, tag="out1_sb")
            nc.vector.tensor_scalar_mul(out1_sb[:], out1[:], 1.0)
            out_sb = smallp.tile([BROWS, C], f32, tag="out_sb")
            nc.vector.scalar_tensor_tensor(
                out=out_sb[:], in0=out2[:], scalar=rden[:], in1=out1_sb[:],
                op0=mybir.AluOpType.mult, op1=mybir.AluOpType.add)
            nc.sync.dma_start(out=out_d[:, :], in_=out_sb[:])
    nc.finalize()
    return nc


def _phase1_nc():
    global _PH1
    if _PH1 is None:
        _PH1 = _build_phase1()
    return _PH1


def _phase2_nc():
    global _PH2
    if _PH2 is None:
        _PH2 = _build_phase2()
    return _PH2


def _run_retry(nc, in_maps):
    try:
        return run_bass_kernel_spmd(nc, in_maps, core_ids=list(range(NC_CORES)))
    except Exception:
        return run_bass_kernel_spmd(nc, in_maps, core_ids=list(range(NC_CORES)))


def _quantize(query_feat, khat_pad):
    """Integer-quantize relu(q) and khat so that fp8e4 DoubleRow matmuls are
    exact and |sim| stays < 8192 (one fp32 binade under BIG)."""
    q32 = np.maximum(query_feat, 0)
    sq = 16.0 / q32.max()
    sk = 15.0 / np.abs(khat_pad).max()
    while True:
        q_int = np.rint(q32 * sq).astype(np.float32)            # 0..16
        k_int = 2.0 * np.rint(khat_pad * sk).astype(np.float32)  # even, |.|<=30
        qn = np.linalg.norm(q_int, axis=1).max()
        kn = np.linalg.norm(k_int, axis=1).max()
        if qn * kn < 8100.0:
            return q_int, k_int
        sq *= 0.95
        sk *= 0.97


def _knn_top32(query_feat, memory_keys):
    """Phase 1 on device + host merge: exact global top-32 indices [B, K]."""
    # ---- host prep: pad + normalize + quantize + rearrange keys ----
    kn = np.sqrt((memory_keys ** 2).sum(axis=1))
    khat = memory_keys * (1.0 / kn)[:, None]
    pad = np.full((NPAD - N, D), -1.0 / np.sqrt(D), np.float32)
    khat_pad = np.concatenate([khat.astype(np.float32), pad], axis=0)
    q_int, k_int = _quantize(query_feat, khat_pad)
    # dims {381..383, 509..511} are repurposed as bias rows: value =
    # BIG + nw*2^-9 where nw = in-window column (0..1023).  The PE sums each
    # DoubleRow (i=0,i=1) pair in ~fp16 before fp32 PSUM, so each pair-sum
    # must be fp16-exact: BIG alone, (a,b) together, c alone.
    q_int[:, [381, 382, 383, 509, 510, 511]] = 0.0
    k_int[:, [381, 382, 383, 509, 510, 511]] = 0.0

    # kT arr: [core][13, 128, 2(h), 2(dc), 2(i), 512(n)]
    #   <- k_int[c*12800 + (2w+h)*512 + n, dc*256 + i*128 + p]; chunk 26 = pad
    k_ext = np.concatenate(
        [k_int.reshape(NC_CORES, 25, 512, D),
         np.full((NC_CORES, 1, 512, D), -30.0, np.float32)], axis=1)
    karr = k_ext.reshape(NC_CORES, 13, 2, 512, 2, 2, 128).transpose(0, 1, 6, 2, 4, 5, 3)
    karr = np.ascontiguousarray(karr)                   # [c, w, p, h, dc, i, n]
    nw = (np.arange(2)[:, None] * 512 + np.arange(512)[None, :]).astype(np.float32)
    karr[:, :, 125, :, 1, 0, :] = 192.0                 # q 128     -> +24576
    karr[:, :, 125, :, 1, 1, :] = 0.0
    karr[:, :, 126, :, 1, 0, :] = np.floor(nw / 256)    # q 2^-1    -> a*2^-1
    karr[:, :, 126, :, 1, 1, :] = np.floor(nw / 16) % 16  # q 2^-5  -> b*2^-5
    karr[:, :, 127, :, 1, 0, :] = nw % 16               # q 2^-9    -> c*2^-9
    karr[:, :, 127, :, 1, 1, :] = 0.0
    karr = karr.astype(ml_dtypes.float8_e4m3)
    # qT arr: [2, 128, 2, 256] <- q_int[q, dc*256+i*128+p]
    qarr = q_int.T.reshape(2, 2, 128, 2, 128).transpose(3, 2, 0, 1, 4)
    qarr = np.ascontiguousarray(qarr)                   # [qt, p, dc, i, q]
    qarr[:, 125, 1, 0, :] = 128.0
    qarr[:, 125, 1, 1, :] = 0.0
    qarr[:, 126, 1, 0, :] = 0.5
    qarr[:, 126, 1, 1, :] = 2.0 ** -5
    qarr[:, 127, 1, 0, :] = 2.0 ** -9
    qarr[:, 127, 1, 1, :] = 0.0
    qarr = qarr.astype(ml_dtypes.float8_e4m3)

    ph1 = _phase1_nc()
    in_maps = [{"kT": karr[c], "qT": qarr} for c in range(NC_CORES)]
    res1 = _run_retry(ph1, in_maps)

    # ---- host: decode candidates, exact re-score, global top-32 ----
    cand_r = []   # row indices
    cand_k = []   # global key indices
    win_base = np.zeros(56, np.int64)       # l1 col -> window base (in-shard)
    for i, (w, lo, hi) in enumerate(_MAX8_LIST):
        win_base[8 * i:8 * i + 8] = w * WIN
    buf2shard = np.zeros(TOPC, np.int64)    # topk buffer col -> shard col
    for (w, lo, hi, r, doff) in _EV_PIECES:
        buf2shard[r * TOPW + doff:r * TOPW + doff + hi - lo] = \
            w * WIN + np.arange(lo, hi)
    rows128 = np.arange(128)
    for c in range(NC_CORES):
        l1 = res1.results[c]["l1"].view(np.uint32)      # [2, 128, 56]
        tk = res1.results[c]["tk"]                      # [2, NREG, 128, 32]
        for qt in range(2):
            # DVE path: packed low-10-bit in-window index
            ks = c * SHARD + win_base[None, :] + (l1[qt] & np.uint32(0x3FF))
            cand_k.append(ks.reshape(-1))
            cand_r.append(np.repeat(qt * 128 + rows128, 56))
            # topk path: flat idx within [16, TOPW] token slab
            for r in range(NREG):
                idx = tk[qt, r, :, 16:32].astype(np.int64).reshape(8, 256)
                p_rel = idx // TOPW
                col = idx % TOPW
                tok = np.arange(8)[:, None]
                rows = qt * 128 + tok * 16 + p_rel
                keys = c * SHARD + buf2shard[r * TOPW + col]
                cand_r.append(rows.reshape(-1))
                cand_k.append(keys.reshape(-1))
    cand_r = np.concatenate(cand_r)
    cand_k = np.concatenate(cand_k)
    keep = cand_k < N
    cand_r = cand_r[keep]
    cand_k = cand_k[keep].astype(np.int64)

    # per-row candidate matrix (padded with key 0 dups; ordered by key index
    # for reference-stable tie-breaking)
    order = np.lexsort((cand_k, cand_r))
    cand_r = cand_r[order]
    cand_k = cand_k[order]
    counts = np.bincount(cand_r, minlength=B)
    maxc = int(counts.max())
    grid = np.zeros((B, maxc), np.int64)
    mask = np.zeros((B, maxc), bool)
    pos = (np.arange(cand_r.size) -
           np.concatenate([[0], np.cumsum(counts)[:-1]])[cand_r])
    grid[cand_r, pos] = cand_k
    mask[cand_r, pos] = True

    q32 = np.maximum(query_feat, 0)
    cand_keys = memory_keys[grid]                       # [B, maxc, D]
    dots = np.einsum("bd,bcd->bc", q32, cand_keys, optimize=True)
    cos = dots / np.maximum(
        np.linalg.norm(q32, axis=1)[:, None] * kn[grid], np.float32(1e-8))
    cos[~mask] = -np.inf
    # dedup: same key may arrive from both paths; keep first occurrence
    dup = np.zeros_like(mask)
    dup[:, 1:] = grid[:, 1:] == grid[:, :-1]
    cos[dup & mask] = -np.inf
    sel = np.argsort(-cos, axis=1, kind="stable")[:, :K]
    return np.take_along_axis(grid, sel, axis=1)        # [256, 32]


def kernel(query_feat, memory_keys, Wq, bq, Wm, bm, Ws, bs, Wc, bc):
    query_feat = np.asarray(query_feat, np.float32)
    memory_keys = np.asarray(memory_keys, np.float32)
    top_idx = _knn_top32(query_feat, memory_keys)
    knn = memory_keys[top_idx]                          # [256, 32, 512]

    # ---- phase 2 (batch sharded) ----
    ph2 = _phase2_nc()
    b16 = ml_dtypes.bfloat16
    bqm = (np.asarray(bq, np.float32) + np.asarray(bm, np.float32)).reshape(A, 1)
    Wqm_a = np.concatenate([np.asarray(Wq, np.float32),
                            np.asarray(Wm, np.float32)], axis=1).astype(b16)
    Ws_a = np.asarray(Ws, np.float32).astype(b16)
    Wc_a = np.asarray(Wc, np.float32).astype(b16)
    S = (np.arange(BROWS)[:, None] == (np.arange(BROWS * K)[None, :] // K)).astype(b16)
    p128 = np.arange(128)
    tj = np.arange(256)
    m256 = ((tj[None, :] % 32) == (4 * (tj[None, :] // 32) + p128[:, None] // 32)
            ).astype(b16)
    qr = np.maximum(query_feat, 0).astype(np.float32)
    in_maps2 = []
    for c in range(NC_CORES):
        rows = slice(c * BROWS, (c + 1) * BROWS)
        knn_c = knn[rows].reshape(BROWS * K, D)
        in_maps2.append({
            "qTr": np.ascontiguousarray(qr[rows].T).astype(b16),
            "knn": knn_c.astype(b16),
            "knnT": np.ascontiguousarray(knn_c.T).astype(b16),
            "Wqm": Wqm_a, "Ws": Ws_a, "bqm": bqm, "Wc": Wc_a,
            "S": S, "m256": m256,
        })
    res2 = _run_retry(ph2, in_maps2)
    out = np.concatenate([res2.results[c]["out"] for c in range(NC_CORES)], axis=0)
    return (out + np.asarray(bc, np.float32)[None, :]).astype(np.float32)
